# revision 14
# baseline (speedup 1.0000x reference)
"""Trainium2 kernel for nn_LJCH1_34548716929306 (ragged_sequence).

Strategy (pure data-parallel over batch, per sharding hint):
  - The dominant cost is the fc0 projection: concat([featContent,
    featDistort, motionFeat]) [16,2048,4864] @ fc0_w.T [4864,128].
    ~637MB fp32 of activations -> memory-regime. Runs on the 8
    NeuronCores, 2 samples per core, as sT = W^T-packed @ xT with
    feature-major (K-major) bf16 layout prepared host-side.
  - Custom Bass/Tile kernel per core (fp16 operands; fp8 E4M3 was tried
    and fails the 2e-2 accuracy gate at 2.3e-2):
      * fc0 weights packed [128(K-part), 38, 128] resident in SBUF
      * x streamed in 4 super-chunks of [128, 38, 1024] fp16 (~10MB),
        double-buffered; one dma_start per super-chunk, per-partition
        contiguous HBM layout (76KB linear blocks)
      * 38-step PSUM fp32 accumulation per 512-col chunk
      * result copied PSUM->SBUF (DVE) and DMA'd out as fp32
  - `reps` builds the same kernel with the whole body inside a
    hardware For_i loop (identical addresses per iteration; weights
    hoisted out). Used by the test harness to measure steady-state
    per-iteration HW time as a slope, cancelling the axon-tunnel RTT.
  - The BiGRU over T=2048 (H=32) and the masked multi-scale softmax
    head are tiny (~0.1% of FLOPs) and sequential; they run in fp32
    numpy on host.
"""

import numpy as np
import ml_dtypes
from concurrent.futures import ThreadPoolExecutor

import concourse.bass as bass
import concourse.bacc as bacc
import concourse.tile as tile
from concourse import mybir
from concourse.bass import ds, ts
from concourse.bass_utils import run_bass_kernel_spmd

B, T = 16, 2048
D_CONTENT, D_DISTORT, D_MOTION = 4096, 512, 256
D = D_CONTENT + D_DISTORT + D_MOTION  # 4864
RED, H = 128, 32
N_CORES = 8
BL = B // N_CORES  # 2 samples per core
NCOL = BL * T  # 4096 columns per core
KT = D // 128  # 38 K-tiles
CH = 1024  # super-chunk columns
N_CH = NCOL // CH  # 4
TIME_INTERVAL = 2
NEG = -1e9

USE_FP8 = False  # fp8 E4M3 fails the 2e-2 gate (2.3e-2); fp16 matches bf16
W_SCALE = 32.0  # only used on the fp8 path

_compiled = {}  # reps -> Bacc


def _build_nc(reps=1):
    dt_in = mybir.dt.float8e4 if USE_FP8 else mybir.dt.float16
    nc = bacc.Bacc(
        "TRN2",
        target_bir_lowering=False,
        debug=False,
        enable_asserts=False,
        num_devices=N_CORES,
    )
    # x4[p, j, k, c] = x[k*128+p, j*CH+c]: per-(partition, chunk) data is
    # one contiguous KT*CH block -> max DMA descriptor efficiency.
    x4 = nc.dram_tensor("x4", [128, N_CH, KT, CH], dt_in, kind="ExternalInput")
    w = nc.dram_tensor("w", [128, KT, RED], dt_in, kind="ExternalInput")
    sT = nc.dram_tensor("sT", [RED, NCOL], mybir.dt.float32, kind="ExternalOutput")

    with tile.TileContext(nc) as tc:
        with tc.tile_pool(name="wp", bufs=1) as wp, \
             tc.tile_pool(name="xp", bufs=2) as xp, \
             tc.tile_pool(name="op", bufs=3) as op, \
             tc.tile_pool(name="pp", bufs=4, space="PSUM") as pp:
            w_sb = wp.tile([128, KT, RED], dt_in)
            nc.sync.dma_start(w_sb[:], w.ap())

            def chunk(j):
                xt = xp.tile([128, KT, CH], dt_in, tag="x")
                nc.sync.dma_start(xt[:], x4.ap()[:, j])
                for h in range(CH // 512):
                    ps = pp.tile([128, 512], mybir.dt.float32, tag="ps")
                    if USE_FP8:
                        for k2 in range(KT // 2):
                            nc.tensor.matmul(
                                ps[:],
                                w_sb[:, 2 * k2 : 2 * k2 + 2, :],
                                xt[:, 2 * k2 : 2 * k2 + 2, ds(h * 512, 512)],
                                start=(k2 == 0),
                                stop=(k2 == KT // 2 - 1),
                                perf_mode=mybir.MatmulPerfMode.DoubleRow,
                            )
                    else:
                        for k in range(KT):
                            nc.tensor.matmul(
                                ps[:],
                                w_sb[:, k, :],
                                xt[:, k : k + 1, ds(h * 512, 512)],
                                start=(k == 0),
                                stop=(k == KT - 1),
                            )
                    ot = op.tile([128, 512], mybir.dt.float32, tag="o")
                    nc.vector.tensor_copy(ot[:], ps[:])
                    nc.sync.dma_start(
                        sT.ap()[:, ds(j * CH + h * 512, 512)], ot[:]
                    )

            if reps == 1:
                for j in range(N_CH):
                    chunk(j)
            else:
                # staggered_reset: no all-engine barrier at the back edge --
                # chunk-0 loads of iteration i+1 overlap the tail compute of
                # iteration i, so the marginal rep is true steady-state.
                with tc.For_i(0, reps, hint_engines=(mybir.EngineType.PE,),
                              staggered_reset=True):
                    for j in range(N_CH):
                        if j:
                            tc.stage_boundary()
                        chunk(j)
    nc.compile()
    return nc


def _get_compiled(reps=1):
    if reps not in _compiled:
        _compiled[reps] = _build_nc(reps)
    return _compiled[reps]


_runners = {}  # reps -> dict(run=..., sharded=..., mesh=..., names...)


def _make_runner(reps=1):
    """Build the sharded PJRT executable once and reuse it across calls.

    Uses bass2jax fast-dispatch (no effects -> C++ dispatch path) and no
    donation so the same device-resident buffers can be re-executed.
    """
    import jax
    from jax.sharding import Mesh, PartitionSpec
    from jax.experimental.shard_map import shard_map
    from concourse import bass2jax
    from concourse import mybir as _mybir

    nc = _get_compiled(reps)
    bass2jax.install_neuronx_cc_hook()

    partition_name = nc.partition_id_tensor.name if nc.partition_id_tensor else None
    in_names, out_names, out_avals = [], [], []
    for alloc in nc.m.functions[0].allocations:
        if not isinstance(alloc, _mybir.MemoryLocationSet):
            continue
        name = alloc.memorylocations[0].name
        if alloc.kind == "ExternalInput":
            if name != partition_name:
                in_names.append(name)
        elif alloc.kind == "ExternalOutput":
            out_names.append(name)
            out_avals.append(
                jax.core.ShapedArray(tuple(alloc.tensor_shape), _mybir.dt.np(alloc.dtype))
            )
    all_in_names = list(in_names) + list(out_names)
    if partition_name is not None:
        all_in_names.append(partition_name)

    def _body(*args):
        operands = list(args)
        if partition_name is not None:
            operands.append(bass2jax.partition_id_tensor())
        outs = bass2jax._bass_exec_p.bind(
            *operands,
            out_avals=tuple(out_avals),
            in_names=tuple(all_in_names),
            out_names=tuple(out_names),
            lowering_input_output_aliases=(),
            sim_require_finite=True,
            sim_require_nnan=True,
            nc=nc,
        )
        return tuple(outs)

    devices = jax.devices()[:N_CORES]
    mesh = Mesh(np.asarray(devices), ("core",))
    n_params, n_outs = len(in_names), len(out_avals)
    in_specs = (PartitionSpec("core"),) * (n_params + n_outs)
    out_specs = (PartitionSpec("core"),) * n_outs

    np_in = ml_dtypes.float8_e4m3 if USE_FP8 else np.float16
    in_shapes = {"x4": (N_CORES * 128, N_CH, KT, CH), "w": (N_CORES * 128, KT, RED)}
    avals = [jax.ShapeDtypeStruct(in_shapes[n], np_in) for n in in_names]
    avals += [
        jax.ShapeDtypeStruct((N_CORES * a.shape[0], *a.shape[1:]), a.dtype)
        for a in out_avals
    ]

    def _jit():
        return jax.jit(
            shard_map(_body, mesh=mesh, in_specs=in_specs, out_specs=out_specs,
                      check_rep=False),
            keep_unused=True,
        )

    try:
        sharded = bass2jax.fast_dispatch_compile(
            lambda: _jit().lower(*avals).compile()
        )
    except Exception:
        sharded = _jit()

    def run(in_maps):
        from jax.sharding import NamedSharding
        sh = NamedSharding(mesh, PartitionSpec("core"))
        concat_in = [
            np.concatenate([np.asarray(m[name]) for m in in_maps], axis=0)
            for name in in_names
        ]
        concat_zeros = [
            np.zeros((N_CORES * a.shape[0], *a.shape[1:]), a.dtype) for a in out_avals
        ]
        out_arrs = sharded(*[jax.device_put(a, sh) for a in concat_in + concat_zeros])
        return [
            {
                name: np.asarray(out_arrs[i]).reshape(N_CORES, *out_avals[i].shape)[c]
                for i, name in enumerate(out_names)
            }
            for c in range(N_CORES)
        ]

    return dict(
        run=run, sharded=sharded, mesh=mesh,
        in_names=in_names, out_names=out_names, out_avals=out_avals,
    )


def _get_runner(reps=1):
    if reps not in _runners:
        _runners[reps] = _make_runner(reps)
    return _runners[reps]


def _run_device(in_maps):
    try:
        return _get_runner(1)["run"](in_maps)
    except Exception:
        return run_bass_kernel_spmd(_get_compiled(1), in_maps, list(range(N_CORES))).results


def _sigmoid(x):
    return 1.0 / (1.0 + np.exp(-x))


def _gru_dir(gi, wh, bh, reverse):
    # gi: [T, B, 3H] precomputed input gates; returns ys [T, B, H]
    Tn, Bn, _ = gi.shape
    whT = wh.T.copy()  # [H, 3H]
    h = np.zeros((Bn, H), np.float32)
    ys = np.empty((Tn, Bn, H), np.float32)
    order = range(Tn - 1, -1, -1) if reverse else range(Tn)
    for t in order:
        g = gi[t]
        gh = h @ whT + bh
        i_r, i_z, i_n = g[:, :H], g[:, H : 2 * H], g[:, 2 * H :]
        h_r, h_z, h_n = gh[:, :H], gh[:, H : 2 * H], gh[:, 2 * H :]
        r = _sigmoid(i_r + h_r)
        z = _sigmoid(i_z + h_z)
        n = np.tanh(i_n + r * h_n)
        h = (1.0 - z) * n + z * h
        ys[t] = h
    return ys


def _conv1d_same(x, w):
    # cross-correlation with zero 'same' padding; x [B,T], w [k]
    k = w.shape[0]
    p = k // 2
    xp = np.pad(x, ((0, 0), (p, p)))
    out = np.zeros_like(x)
    for j in range(k):
        out += w[j] * xp[:, j : j + x.shape[1]]
    return out


def _pack_weights(fc0_w):
    # w_packed[p, k, m] = fc0_w[m, k*128+p] * scale  (lhsT k-tiles)
    np_in = ml_dtypes.float8_e4m3 if USE_FP8 else np.float16
    scale = W_SCALE if USE_FP8 else 1.0
    return np.ascontiguousarray(
        (fc0_w.T * scale).reshape(KT, 128, RED).transpose(1, 0, 2)
    ).astype(np_in)


def _make_in_maps(inputs):
    fC = np.asarray(inputs["featContent"], np.float32)
    fD = np.asarray(inputs["featDistort"], np.float32)
    mF = np.asarray(inputs["motionFeat"], np.float32)
    fc0_w = np.asarray(inputs["fc0_w"], np.float32)
    w_np = _pack_weights(fc0_w)
    np_in = ml_dtypes.float8_e4m3 if USE_FP8 else np.float16

    def build(c):
        sl = slice(c * BL, (c + 1) * BL)
        xT = np.empty((D, NCOL), np_in)
        xT[:D_CONTENT] = fC[sl].reshape(NCOL, D_CONTENT).T
        xT[D_CONTENT : D_CONTENT + D_DISTORT] = fD[sl].reshape(NCOL, D_DISTORT).T
        xT[D_CONTENT + D_DISTORT :] = mF[sl].reshape(NCOL, D_MOTION).T
        # x4[p, j, k, c] = xT[k*128+p, j*CH+c]
        x4 = np.ascontiguousarray(
            xT.reshape(KT, 128, N_CH, CH).transpose(1, 2, 0, 3)
        )
        return {"x4": x4, "w": w_np}

    with ThreadPoolExecutor(N_CORES) as ex:
        return list(ex.map(build, range(N_CORES)))


def kernel(**inputs):
    inputLength = np.asarray(inputs["inputLength"])
    fc0_b = np.asarray(inputs["fc0_b"], np.float32)

    in_maps = _make_in_maps(inputs)
    results = _run_device(in_maps)

    unscale = np.float32(1.0 / W_SCALE) if USE_FP8 else np.float32(1.0)
    scores = np.empty((B, T, RED), np.float32)
    for c in range(N_CORES):
        sT = results[c]["sT"]  # [RED, BL*T]
        scores[c * BL : (c + 1) * BL] = (
            sT.T.reshape(BL, T, RED).astype(np.float32) * unscale
        )
    scores += fc0_b

    # BiGRU (fp32 host)
    x_tbd = scores.transpose(1, 0, 2)  # [T,B,RED]
    gi_f = x_tbd @ np.asarray(inputs["gru_wi_f"], np.float32).T + np.asarray(
        inputs["gru_bi_f"], np.float32
    )
    gi_b = x_tbd @ np.asarray(inputs["gru_wi_b"], np.float32).T + np.asarray(
        inputs["gru_bi_b"], np.float32
    )
    yf = _gru_dir(gi_f, np.asarray(inputs["gru_wh_f"], np.float32),
                  np.asarray(inputs["gru_bh_f"], np.float32), reverse=False)
    yb = _gru_dir(gi_b, np.asarray(inputs["gru_wh_b"], np.float32),
                  np.asarray(inputs["gru_bh_b"], np.float32), reverse=True)
    outputs = np.concatenate([yf, yb], -1).transpose(1, 0, 2)  # [B,T,2H]

    q_w = np.asarray(inputs["q_w"], np.float32)
    q_b = np.asarray(inputs["q_b"], np.float32)
    q = (outputs @ q_w.T + q_b)[..., 0]  # [B,T]

    lengths = inputLength.astype(np.int64) - 2 * (TIME_INTERVAL // 2) - 1
    mask = np.arange(T)[None, :] < lengths[:, None]
    qm = np.where(mask, q, 0.0).astype(np.float32)

    total = np.zeros((B,), np.float32)
    for wk in ("w1", "w2", "w3"):
        w = np.asarray(inputs[wk], np.float32)
        logits = np.where(mask, _conv1d_same(qm, w), NEG).astype(np.float32)
        m = logits.max(-1, keepdims=True)
        e = np.exp(logits - m)
        sm = e / e.sum(-1, keepdims=True)
        total = total + (sm * qm).sum(-1)
    return (total / 3.0)[:, None].astype(np.float32)


# revision 23
# speedup vs baseline: 1.3426x; 1.3426x over previous
"""Trainium2 kernel for nn_LJCH1_34548716929306 (ragged_sequence).

Strategy (pure data-parallel over batch, per sharding hint):
  - The dominant cost is the fc0 projection: concat([featContent,
    featDistort, motionFeat]) [16,2048,4864] @ fc0_w.T [4864,128].
    ~637MB fp32 of activations -> memory-regime. Runs on the 8
    NeuronCores, 2 samples per core, as sT = W^T-packed @ xT with
    feature-major (K-major) bf16 layout prepared host-side.
  - Custom Bass/Tile kernel per core (fp16 operands; fp8 E4M3 was tried
    and fails the 2e-2 accuracy gate at 2.3e-2):
      * fc0 weights packed [128(K-part), 38, 128] resident in SBUF
      * x streamed in 4 super-chunks of [128, 38, 1024] fp16 (~10MB),
        double-buffered; one dma_start per super-chunk, per-partition
        contiguous HBM layout (76KB linear blocks)
      * 38-step PSUM fp32 accumulation per 512-col chunk
      * result copied PSUM->SBUF (DVE) and DMA'd out as fp32
  - `reps` builds the same kernel with the whole body inside a
    hardware For_i loop (identical addresses per iteration; weights
    hoisted out). Used by the test harness to measure steady-state
    per-iteration HW time as a slope, cancelling the axon-tunnel RTT.
  - The BiGRU over T=2048 (H=32) and the masked multi-scale softmax
    head are tiny (~0.1% of FLOPs) and sequential; they run in fp32
    numpy on host.
"""

import numpy as np
import ml_dtypes
from concurrent.futures import ThreadPoolExecutor

import concourse.bass as bass
import concourse.bacc as bacc
import concourse.tile as tile
from concourse import mybir
from concourse.bass import ds, ts
from concourse.bass_utils import run_bass_kernel_spmd

B, T = 16, 2048
D_CONTENT, D_DISTORT, D_MOTION = 4096, 512, 256
D = D_CONTENT + D_DISTORT + D_MOTION  # 4864
RED, H = 128, 32
N_CORES = 8
BL = B // N_CORES  # 2 samples per core
NCOL = BL * T  # 4096 columns per core
KT = D // 128  # 38 K-tiles
CH = 512  # super-chunk columns
N_CH = NCOL // CH  # 8
TIME_INTERVAL = 2
NEG = -1e9

USE_FP8 = False  # fp8 E4M3 fails the 2e-2 gate (2.3e-2); fp16 matches bf16
W_SCALE = 32.0  # only used on the fp8 path

_compiled = {}  # reps -> Bacc


def _build_nc(reps=1, dt_in=None, staggered=False):
    if dt_in is None:
        dt_in = mybir.dt.float8e4 if USE_FP8 else mybir.dt.float16
    nc = bacc.Bacc(
        "TRN2",
        target_bir_lowering=False,
        debug=False,
        enable_asserts=False,
        num_devices=N_CORES,
    )
    # x4[p, j, k, c] = x[k*128+p, j*CH+c]: per-(partition, chunk) data is
    # one contiguous KT*CH block -> max DMA descriptor efficiency.
    x4 = nc.dram_tensor("x4", [128, N_CH, KT, CH], dt_in, kind="ExternalInput")
    w = nc.dram_tensor("w", [128, KT, RED], dt_in, kind="ExternalInput")
    sT = nc.dram_tensor("sT", [RED, NCOL], mybir.dt.float32, kind="ExternalOutput")

    with tile.TileContext(nc) as tc:
        with tc.tile_pool(name="wp", bufs=1) as wp, \
             tc.tile_pool(name="x0p", bufs=1) as x0p, \
             tc.tile_pool(name="xp", bufs=3) as xp, \
             tc.tile_pool(name="op", bufs=3) as op, \
             tc.tile_pool(name="pp", bufs=4, space="PSUM") as pp:
            w_sb = wp.tile([128, KT, RED], dt_in)
            nc.sync.dma_start(w_sb[:], w.ap())

            def compute(xt, j):
                ps = pp.tile([128, 512], mybir.dt.float32, tag="ps")
                if dt_in == mybir.dt.float8e4:
                    for k2 in range(KT // 2):
                        nc.tensor.matmul(
                            ps[:],
                            w_sb[:, 2 * k2 : 2 * k2 + 2, :],
                            xt[:, 2 * k2 : 2 * k2 + 2, :],
                            start=(k2 == 0),
                            stop=(k2 == KT // 2 - 1),
                            perf_mode=mybir.MatmulPerfMode.DoubleRow,
                        )
                else:
                    for k in range(KT):
                        nc.tensor.matmul(
                            ps[:],
                            w_sb[:, k, :],
                            xt[:, k : k + 1, :],
                            start=(k == 0),
                            stop=(k == KT - 1),
                        )
                ot = op.tile([128, 512], mybir.dt.float32, tag="o")
                nc.vector.tensor_copy(ot[:], ps[:])
                nc.sync.dma_start(sT.ap()[:, ds(j * CH, CH)], ot[:])

            def chunk(j):
                xt = xp.tile([128, KT, CH], dt_in, tag="x")
                nc.sync.dma_start(xt[:], x4.ap()[:, j])
                compute(xt, j)

            if reps == 1:
                for j in range(N_CH):
                    chunk(j)
            else:
                # Software-pipeline chunk 0 across the back edge: its tile is
                # loaded before the loop and refreshed at the END of each
                # iteration (overlapping chunks 1..N-1), so the iteration
                # never stalls on its first load. Same per-iteration HBM
                # traffic (N_CH chunk loads), no start-of-body DMA ramp.
                xt0 = x0p.tile([128, KT, CH], dt_in)
                nc.sync.dma_start(xt0[:], x4.ap()[:, 0])
                with tc.For_i(0, reps, hint_engines=(mybir.EngineType.PE,)):
                    compute(xt0, 0)
                    for j in range(1, N_CH):
                        chunk(j)
                    nc.sync.dma_start(xt0[:], x4.ap()[:, 0])
    nc.compile()
    return nc


def _get_compiled(reps=1, dt_in=None, staggered=False):
    key = (reps, dt_in, staggered)
    if key not in _compiled:
        _compiled[key] = _build_nc(reps, dt_in=dt_in, staggered=staggered)
    return _compiled[key]


_runners = {}  # reps -> dict(run=..., sharded=..., mesh=..., names...)


def _make_runner(reps=1, dt_in=None, staggered=False):
    """Build the sharded PJRT executable once and reuse it across calls.

    Uses bass2jax fast-dispatch (no effects -> C++ dispatch path) and no
    donation so the same device-resident buffers can be re-executed.
    """
    import jax
    from jax.sharding import Mesh, PartitionSpec
    from jax.experimental.shard_map import shard_map
    from concourse import bass2jax
    from concourse import mybir as _mybir

    nc = _get_compiled(reps, dt_in=dt_in, staggered=staggered)
    bass2jax.install_neuronx_cc_hook()

    partition_name = nc.partition_id_tensor.name if nc.partition_id_tensor else None
    in_names, out_names, out_avals = [], [], []
    for alloc in nc.m.functions[0].allocations:
        if not isinstance(alloc, _mybir.MemoryLocationSet):
            continue
        name = alloc.memorylocations[0].name
        if alloc.kind == "ExternalInput":
            if name != partition_name:
                in_names.append(name)
        elif alloc.kind == "ExternalOutput":
            out_names.append(name)
            out_avals.append(
                jax.core.ShapedArray(tuple(alloc.tensor_shape), _mybir.dt.np(alloc.dtype))
            )
    all_in_names = list(in_names) + list(out_names)
    if partition_name is not None:
        all_in_names.append(partition_name)

    def _body(*args):
        operands = list(args)
        if partition_name is not None:
            operands.append(bass2jax.partition_id_tensor())
        outs = bass2jax._bass_exec_p.bind(
            *operands,
            out_avals=tuple(out_avals),
            in_names=tuple(all_in_names),
            out_names=tuple(out_names),
            lowering_input_output_aliases=(),
            sim_require_finite=True,
            sim_require_nnan=True,
            nc=nc,
        )
        return tuple(outs)

    devices = jax.devices()[:N_CORES]
    mesh = Mesh(np.asarray(devices), ("core",))
    n_params, n_outs = len(in_names), len(out_avals)
    in_specs = (PartitionSpec("core"),) * (n_params + n_outs)
    out_specs = (PartitionSpec("core"),) * n_outs

    _np_map = {
        mybir.dt.float8e4: ml_dtypes.float8_e4m3,
        mybir.dt.bfloat16: ml_dtypes.bfloat16,
        mybir.dt.float16: np.float16,
        None: ml_dtypes.float8_e4m3 if USE_FP8 else np.float16,
    }
    np_in = _np_map[dt_in]
    in_shapes = {"x4": (N_CORES * 128, N_CH, KT, CH), "w": (N_CORES * 128, KT, RED)}
    avals = [jax.ShapeDtypeStruct(in_shapes[n], np_in) for n in in_names]
    avals += [
        jax.ShapeDtypeStruct((N_CORES * a.shape[0], *a.shape[1:]), a.dtype)
        for a in out_avals
    ]

    def _jit():
        return jax.jit(
            shard_map(_body, mesh=mesh, in_specs=in_specs, out_specs=out_specs,
                      check_rep=False),
            keep_unused=True,
        )

    try:
        sharded = bass2jax.fast_dispatch_compile(
            lambda: _jit().lower(*avals).compile()
        )
    except Exception:
        sharded = _jit()

    def run(in_maps):
        from jax.sharding import NamedSharding
        sh = NamedSharding(mesh, PartitionSpec("core"))
        concat_in = [
            np.concatenate([np.asarray(m[name]) for m in in_maps], axis=0)
            for name in in_names
        ]
        concat_zeros = [
            np.zeros((N_CORES * a.shape[0], *a.shape[1:]), a.dtype) for a in out_avals
        ]
        out_arrs = sharded(*[jax.device_put(a, sh) for a in concat_in + concat_zeros])
        return [
            {
                name: np.asarray(out_arrs[i]).reshape(N_CORES, *out_avals[i].shape)[c]
                for i, name in enumerate(out_names)
            }
            for c in range(N_CORES)
        ]

    return dict(
        run=run, sharded=sharded, mesh=mesh,
        in_names=in_names, out_names=out_names, out_avals=out_avals,
    )


def _get_runner(reps=1, dt_in=None, staggered=False):
    key = (reps, dt_in, staggered)
    if key not in _runners:
        _runners[key] = _make_runner(reps, dt_in=dt_in, staggered=staggered)
    return _runners[key]


def _run_device(in_maps):
    try:
        return _get_runner(1)["run"](in_maps)
    except Exception:
        return run_bass_kernel_spmd(_get_compiled(1), in_maps, list(range(N_CORES))).results


def _sigmoid(x):
    return 1.0 / (1.0 + np.exp(-x))


def _gru_dir(gi, wh, bh, reverse):
    # gi: [T, B, 3H] precomputed input gates; returns ys [T, B, H]
    Tn, Bn, _ = gi.shape
    whT = wh.T.copy()  # [H, 3H]
    h = np.zeros((Bn, H), np.float32)
    ys = np.empty((Tn, Bn, H), np.float32)
    order = range(Tn - 1, -1, -1) if reverse else range(Tn)
    for t in order:
        g = gi[t]
        gh = h @ whT + bh
        i_r, i_z, i_n = g[:, :H], g[:, H : 2 * H], g[:, 2 * H :]
        h_r, h_z, h_n = gh[:, :H], gh[:, H : 2 * H], gh[:, 2 * H :]
        r = _sigmoid(i_r + h_r)
        z = _sigmoid(i_z + h_z)
        n = np.tanh(i_n + r * h_n)
        h = (1.0 - z) * n + z * h
        ys[t] = h
    return ys


def _conv1d_same(x, w):
    # cross-correlation with zero 'same' padding; x [B,T], w [k]
    k = w.shape[0]
    p = k // 2
    xp = np.pad(x, ((0, 0), (p, p)))
    out = np.zeros_like(x)
    for j in range(k):
        out += w[j] * xp[:, j : j + x.shape[1]]
    return out


def _pack_weights(fc0_w):
    # w_packed[p, k, m] = fc0_w[m, k*128+p] * scale  (lhsT k-tiles)
    np_in = ml_dtypes.float8_e4m3 if USE_FP8 else np.float16
    scale = W_SCALE if USE_FP8 else 1.0
    return np.ascontiguousarray(
        (fc0_w.T * scale).reshape(KT, 128, RED).transpose(1, 0, 2)
    ).astype(np_in)


def _make_in_maps(inputs):
    fC = np.asarray(inputs["featContent"], np.float32)
    fD = np.asarray(inputs["featDistort"], np.float32)
    mF = np.asarray(inputs["motionFeat"], np.float32)
    fc0_w = np.asarray(inputs["fc0_w"], np.float32)
    w_np = _pack_weights(fc0_w)
    np_in = ml_dtypes.float8_e4m3 if USE_FP8 else np.float16

    def build(c):
        sl = slice(c * BL, (c + 1) * BL)
        xT = np.empty((D, NCOL), np_in)
        xT[:D_CONTENT] = fC[sl].reshape(NCOL, D_CONTENT).T
        xT[D_CONTENT : D_CONTENT + D_DISTORT] = fD[sl].reshape(NCOL, D_DISTORT).T
        xT[D_CONTENT + D_DISTORT :] = mF[sl].reshape(NCOL, D_MOTION).T
        # x4[p, j, k, c] = xT[k*128+p, j*CH+c]
        x4 = np.ascontiguousarray(
            xT.reshape(KT, 128, N_CH, CH).transpose(1, 2, 0, 3)
        )
        return {"x4": x4, "w": w_np}

    with ThreadPoolExecutor(N_CORES) as ex:
        return list(ex.map(build, range(N_CORES)))


def kernel(**inputs):
    inputLength = np.asarray(inputs["inputLength"])
    fc0_b = np.asarray(inputs["fc0_b"], np.float32)

    in_maps = _make_in_maps(inputs)
    results = _run_device(in_maps)

    unscale = np.float32(1.0 / W_SCALE) if USE_FP8 else np.float32(1.0)
    scores = np.empty((B, T, RED), np.float32)
    for c in range(N_CORES):
        sT = results[c]["sT"]  # [RED, BL*T]
        scores[c * BL : (c + 1) * BL] = (
            sT.T.reshape(BL, T, RED).astype(np.float32) * unscale
        )
    scores += fc0_b

    # BiGRU (fp32 host)
    x_tbd = scores.transpose(1, 0, 2)  # [T,B,RED]
    gi_f = x_tbd @ np.asarray(inputs["gru_wi_f"], np.float32).T + np.asarray(
        inputs["gru_bi_f"], np.float32
    )
    gi_b = x_tbd @ np.asarray(inputs["gru_wi_b"], np.float32).T + np.asarray(
        inputs["gru_bi_b"], np.float32
    )
    yf = _gru_dir(gi_f, np.asarray(inputs["gru_wh_f"], np.float32),
                  np.asarray(inputs["gru_bh_f"], np.float32), reverse=False)
    yb = _gru_dir(gi_b, np.asarray(inputs["gru_wh_b"], np.float32),
                  np.asarray(inputs["gru_bh_b"], np.float32), reverse=True)
    outputs = np.concatenate([yf, yb], -1).transpose(1, 0, 2)  # [B,T,2H]

    q_w = np.asarray(inputs["q_w"], np.float32)
    q_b = np.asarray(inputs["q_b"], np.float32)
    q = (outputs @ q_w.T + q_b)[..., 0]  # [B,T]

    lengths = inputLength.astype(np.int64) - 2 * (TIME_INTERVAL // 2) - 1
    mask = np.arange(T)[None, :] < lengths[:, None]
    qm = np.where(mask, q, 0.0).astype(np.float32)

    total = np.zeros((B,), np.float32)
    for wk in ("w1", "w2", "w3"):
        w = np.asarray(inputs[wk], np.float32)
        logits = np.where(mask, _conv1d_same(qm, w), NEG).astype(np.float32)
        m = logits.max(-1, keepdims=True)
        e = np.exp(logits - m)
        sm = e / e.sum(-1, keepdims=True)
        total = total + (sm * qm).sum(-1)
    return (total / 3.0)[:, None].astype(np.float32)


# revision 30
# speedup vs baseline: 1.4247x; 1.0611x over previous
"""Trainium2 kernel for nn_LJCH1_34548716929306 (ragged_sequence).

Strategy (pure data-parallel over batch, per sharding hint):
  - The dominant cost is the fc0 projection: concat([featContent,
    featDistort, motionFeat]) [16,2048,4864] @ fc0_w.T [4864,128].
    ~637MB fp32 of activations -> memory-regime. Runs on the 8
    NeuronCores, 2 samples per core, as sT = W^T-packed @ xT with
    feature-major (K-major) bf16 layout prepared host-side.
  - Custom Bass/Tile kernel per core (fp16 operands; fp8 E4M3 was tried
    and fails the 2e-2 accuracy gate at 2.3e-2):
      * fc0 weights packed [128(K-part), 38, 128] resident in SBUF
      * x streamed in 8 chunks of [128, 38, 512] fp16 (~5MB), triple
        buffered; one dma_start per chunk, alternating between the two
        HWDGE rings (sync/scalar); per-partition contiguous HBM layout
        (38KB linear blocks)
      * 38-step PSUM fp32 accumulation per 512-col chunk
      * result copied PSUM->SBUF (DVE) and DMA'd out as fp32
      * in the timing loop, chunk 0 is software-pipelined across the
        back edge (persistent tile refreshed at end of body)
  - `reps` builds the same kernel with the whole body inside a
    hardware For_i loop (identical addresses per iteration; weights
    hoisted out). Used by the test harness to measure steady-state
    per-iteration HW time as a slope, cancelling the axon-tunnel RTT.
  - The BiGRU over T=2048 (H=32) and the masked multi-scale softmax
    head are tiny (~0.1% of FLOPs) and sequential; they run in fp32
    numpy on host.
"""

import numpy as np
import ml_dtypes
from concurrent.futures import ThreadPoolExecutor

import concourse.bass as bass
import concourse.bacc as bacc
import concourse.tile as tile
from concourse import mybir
from concourse.bass import ds, ts
from concourse.bass_utils import run_bass_kernel_spmd

B, T = 16, 2048
D_CONTENT, D_DISTORT, D_MOTION = 4096, 512, 256
D = D_CONTENT + D_DISTORT + D_MOTION  # 4864
RED, H = 128, 32
N_CORES = 8
BL = B // N_CORES  # 2 samples per core
NCOL = BL * T  # 4096 columns per core
KT = D // 128  # 38 K-tiles
CH = 512  # super-chunk columns
N_CH = NCOL // CH  # 8
TIME_INTERVAL = 2
NEG = -1e9

USE_FP8 = False  # fp8 E4M3 fails the 2e-2 gate (2.3e-2); fp16 matches bf16
W_SCALE = 32.0  # only used on the fp8 path

_compiled = {}  # reps -> Bacc


def _build_nc(reps=1, dt_in=None, staggered=False, mode="full"):
    if dt_in is None:
        dt_in = mybir.dt.float8e4 if USE_FP8 else mybir.dt.float16
    nc = bacc.Bacc(
        "TRN2",
        target_bir_lowering=False,
        debug=False,
        enable_asserts=False,
        num_devices=N_CORES,
    )
    # x4[p, j, k, c] = x[k*128+p, j*CH+c]: per-(partition, chunk) data is
    # one contiguous KT*CH block -> max DMA descriptor efficiency.
    x4 = nc.dram_tensor("x4", [128, N_CH, KT, CH], dt_in, kind="ExternalInput")
    w = nc.dram_tensor("w", [128, KT, RED], dt_in, kind="ExternalInput")
    sT = nc.dram_tensor("sT", [RED, NCOL], mybir.dt.float32, kind="ExternalOutput")

    with tile.TileContext(nc) as tc:
        with tc.tile_pool(name="wp", bufs=1) as wp, \
             tc.tile_pool(name="x0p", bufs=1) as x0p, \
             tc.tile_pool(name="xp", bufs=3) as xp, \
             tc.tile_pool(name="op", bufs=3) as op, \
             tc.tile_pool(name="pp", bufs=4, space="PSUM") as pp:
            w_sb = wp.tile([128, KT, RED], dt_in)
            nc.sync.dma_start(w_sb[:], w.ap())

            def load_engine(j):
                # alternate chunk loads across both HWDGE rings
                return nc.sync if j % 2 == 0 else nc.scalar

            def compute(xt, j):
                ps = pp.tile([128, 512], mybir.dt.float32, tag="ps")
                if dt_in == mybir.dt.float8e4:
                    for k2 in range(KT // 2):
                        nc.tensor.matmul(
                            ps[:],
                            w_sb[:, 2 * k2 : 2 * k2 + 2, :],
                            xt[:, 2 * k2 : 2 * k2 + 2, :],
                            start=(k2 == 0),
                            stop=(k2 == KT // 2 - 1),
                            perf_mode=mybir.MatmulPerfMode.DoubleRow,
                        )
                else:
                    for k in range(KT):
                        nc.tensor.matmul(
                            ps[:],
                            w_sb[:, k, :],
                            xt[:, k : k + 1, :],
                            start=(k == 0),
                            stop=(k == KT - 1),
                        )
                ot = op.tile([128, 512], mybir.dt.float32, tag="o")
                nc.vector.tensor_copy(ot[:], ps[:])
                nc.sync.dma_start(sT.ap()[:, ds(j * CH, CH)], ot[:])

            def chunk(j):
                xt = xp.tile([128, KT, CH], dt_in, tag="x")
                load_engine(j).dma_start(xt[:], x4.ap()[:, j])
                compute(xt, j)

            if mode == "mm":
                # diagnostic: matmuls only, from one resident chunk
                xt0 = x0p.tile([128, KT, CH], dt_in)
                nc.sync.dma_start(xt0[:], x4.ap()[:, 0])
                with tc.For_i(0, reps, hint_engines=(mybir.EngineType.PE,)):
                    for j in range(N_CH):
                        compute(xt0, j)
            elif mode == "dma":
                # diagnostic: chunk loads only + one anchor compute
                with tc.For_i(0, reps, hint_engines=(mybir.EngineType.PE,)):
                    last = None
                    for j in range(N_CH):
                        xt = xp.tile([128, KT, CH], dt_in, tag="x")
                        load_engine(j).dma_start(xt[:], x4.ap()[:, j])
                        last = xt
                    compute(last, 0)
            elif reps == 1:
                for j in range(N_CH):
                    chunk(j)
            else:
                # Software-pipeline chunk 0 across the back edge: its tile is
                # loaded before the loop and refreshed at the END of each
                # iteration (overlapping chunks 1..N-1), so the iteration
                # never stalls on its first load. Same per-iteration HBM
                # traffic (N_CH chunk loads), no start-of-body DMA ramp.
                xt0 = x0p.tile([128, KT, CH], dt_in)
                nc.sync.dma_start(xt0[:], x4.ap()[:, 0])
                with tc.For_i(0, reps, hint_engines=(mybir.EngineType.PE,)):
                    compute(xt0, 0)
                    for j in range(1, N_CH):
                        chunk(j)
                    nc.sync.dma_start(xt0[:], x4.ap()[:, 0])
    nc.compile()
    return nc


def _get_compiled(reps=1, dt_in=None, staggered=False, mode="full"):
    key = (reps, dt_in, staggered, mode)
    if key not in _compiled:
        _compiled[key] = _build_nc(reps, dt_in=dt_in, staggered=staggered, mode=mode)
    return _compiled[key]


_runners = {}  # reps -> dict(run=..., sharded=..., mesh=..., names...)


def _make_runner(reps=1, dt_in=None, staggered=False, mode="full"):
    """Build the sharded PJRT executable once and reuse it across calls.

    Uses bass2jax fast-dispatch (no effects -> C++ dispatch path) and no
    donation so the same device-resident buffers can be re-executed.
    """
    import jax
    from jax.sharding import Mesh, PartitionSpec
    from jax.experimental.shard_map import shard_map
    from concourse import bass2jax
    from concourse import mybir as _mybir

    nc = _get_compiled(reps, dt_in=dt_in, staggered=staggered, mode=mode)
    bass2jax.install_neuronx_cc_hook()

    partition_name = nc.partition_id_tensor.name if nc.partition_id_tensor else None
    in_names, out_names, out_avals = [], [], []
    for alloc in nc.m.functions[0].allocations:
        if not isinstance(alloc, _mybir.MemoryLocationSet):
            continue
        name = alloc.memorylocations[0].name
        if alloc.kind == "ExternalInput":
            if name != partition_name:
                in_names.append(name)
        elif alloc.kind == "ExternalOutput":
            out_names.append(name)
            out_avals.append(
                jax.core.ShapedArray(tuple(alloc.tensor_shape), _mybir.dt.np(alloc.dtype))
            )
    all_in_names = list(in_names) + list(out_names)
    if partition_name is not None:
        all_in_names.append(partition_name)

    def _body(*args):
        operands = list(args)
        if partition_name is not None:
            operands.append(bass2jax.partition_id_tensor())
        outs = bass2jax._bass_exec_p.bind(
            *operands,
            out_avals=tuple(out_avals),
            in_names=tuple(all_in_names),
            out_names=tuple(out_names),
            lowering_input_output_aliases=(),
            sim_require_finite=True,
            sim_require_nnan=True,
            nc=nc,
        )
        return tuple(outs)

    devices = jax.devices()[:N_CORES]
    mesh = Mesh(np.asarray(devices), ("core",))
    n_params, n_outs = len(in_names), len(out_avals)
    in_specs = (PartitionSpec("core"),) * (n_params + n_outs)
    out_specs = (PartitionSpec("core"),) * n_outs

    _np_map = {
        mybir.dt.float8e4: ml_dtypes.float8_e4m3,
        mybir.dt.bfloat16: ml_dtypes.bfloat16,
        mybir.dt.float16: np.float16,
        None: ml_dtypes.float8_e4m3 if USE_FP8 else np.float16,
    }
    np_in = _np_map[dt_in]
    in_shapes = {"x4": (N_CORES * 128, N_CH, KT, CH), "w": (N_CORES * 128, KT, RED)}
    avals = [jax.ShapeDtypeStruct(in_shapes[n], np_in) for n in in_names]
    avals += [
        jax.ShapeDtypeStruct((N_CORES * a.shape[0], *a.shape[1:]), a.dtype)
        for a in out_avals
    ]

    def _jit():
        return jax.jit(
            shard_map(_body, mesh=mesh, in_specs=in_specs, out_specs=out_specs,
                      check_rep=False),
            keep_unused=True,
        )

    try:
        sharded = bass2jax.fast_dispatch_compile(
            lambda: _jit().lower(*avals).compile()
        )
    except Exception:
        sharded = _jit()

    def run(in_maps):
        from jax.sharding import NamedSharding
        sh = NamedSharding(mesh, PartitionSpec("core"))
        concat_in = [
            np.concatenate([np.asarray(m[name]) for m in in_maps], axis=0)
            for name in in_names
        ]
        concat_zeros = [
            np.zeros((N_CORES * a.shape[0], *a.shape[1:]), a.dtype) for a in out_avals
        ]
        out_arrs = sharded(*[jax.device_put(a, sh) for a in concat_in + concat_zeros])
        return [
            {
                name: np.asarray(out_arrs[i]).reshape(N_CORES, *out_avals[i].shape)[c]
                for i, name in enumerate(out_names)
            }
            for c in range(N_CORES)
        ]

    return dict(
        run=run, sharded=sharded, mesh=mesh,
        in_names=in_names, out_names=out_names, out_avals=out_avals,
    )


def _get_runner(reps=1, dt_in=None, staggered=False, mode="full"):
    key = (reps, dt_in, staggered, mode)
    if key not in _runners:
        _runners[key] = _make_runner(reps, dt_in=dt_in, staggered=staggered, mode=mode)
    return _runners[key]


def _run_device(in_maps):
    try:
        return _get_runner(1)["run"](in_maps)
    except Exception:
        return run_bass_kernel_spmd(_get_compiled(1), in_maps, list(range(N_CORES))).results


def _sigmoid(x):
    return 1.0 / (1.0 + np.exp(-x))


def _gru_dir(gi, wh, bh, reverse):
    # gi: [T, B, 3H] precomputed input gates; returns ys [T, B, H]
    Tn, Bn, _ = gi.shape
    whT = wh.T.copy()  # [H, 3H]
    h = np.zeros((Bn, H), np.float32)
    ys = np.empty((Tn, Bn, H), np.float32)
    order = range(Tn - 1, -1, -1) if reverse else range(Tn)
    for t in order:
        g = gi[t]
        gh = h @ whT + bh
        i_r, i_z, i_n = g[:, :H], g[:, H : 2 * H], g[:, 2 * H :]
        h_r, h_z, h_n = gh[:, :H], gh[:, H : 2 * H], gh[:, 2 * H :]
        r = _sigmoid(i_r + h_r)
        z = _sigmoid(i_z + h_z)
        n = np.tanh(i_n + r * h_n)
        h = (1.0 - z) * n + z * h
        ys[t] = h
    return ys


def _conv1d_same(x, w):
    # cross-correlation with zero 'same' padding; x [B,T], w [k]
    k = w.shape[0]
    p = k // 2
    xp = np.pad(x, ((0, 0), (p, p)))
    out = np.zeros_like(x)
    for j in range(k):
        out += w[j] * xp[:, j : j + x.shape[1]]
    return out


def _pack_weights(fc0_w):
    # w_packed[p, k, m] = fc0_w[m, k*128+p] * scale  (lhsT k-tiles)
    np_in = ml_dtypes.float8_e4m3 if USE_FP8 else np.float16
    scale = W_SCALE if USE_FP8 else 1.0
    return np.ascontiguousarray(
        (fc0_w.T * scale).reshape(KT, 128, RED).transpose(1, 0, 2)
    ).astype(np_in)


def _make_in_maps(inputs):
    fC = np.asarray(inputs["featContent"], np.float32)
    fD = np.asarray(inputs["featDistort"], np.float32)
    mF = np.asarray(inputs["motionFeat"], np.float32)
    fc0_w = np.asarray(inputs["fc0_w"], np.float32)
    w_np = _pack_weights(fc0_w)
    np_in = ml_dtypes.float8_e4m3 if USE_FP8 else np.float16

    def build(c):
        sl = slice(c * BL, (c + 1) * BL)
        xT = np.empty((D, NCOL), np_in)
        xT[:D_CONTENT] = fC[sl].reshape(NCOL, D_CONTENT).T
        xT[D_CONTENT : D_CONTENT + D_DISTORT] = fD[sl].reshape(NCOL, D_DISTORT).T
        xT[D_CONTENT + D_DISTORT :] = mF[sl].reshape(NCOL, D_MOTION).T
        # x4[p, j, k, c] = xT[k*128+p, j*CH+c]
        x4 = np.ascontiguousarray(
            xT.reshape(KT, 128, N_CH, CH).transpose(1, 2, 0, 3)
        )
        return {"x4": x4, "w": w_np}

    with ThreadPoolExecutor(N_CORES) as ex:
        return list(ex.map(build, range(N_CORES)))


def kernel(**inputs):
    inputLength = np.asarray(inputs["inputLength"])
    fc0_b = np.asarray(inputs["fc0_b"], np.float32)

    in_maps = _make_in_maps(inputs)
    results = _run_device(in_maps)

    unscale = np.float32(1.0 / W_SCALE) if USE_FP8 else np.float32(1.0)
    scores = np.empty((B, T, RED), np.float32)
    for c in range(N_CORES):
        sT = results[c]["sT"]  # [RED, BL*T]
        scores[c * BL : (c + 1) * BL] = (
            sT.T.reshape(BL, T, RED).astype(np.float32) * unscale
        )
    scores += fc0_b

    # BiGRU (fp32 host)
    x_tbd = scores.transpose(1, 0, 2)  # [T,B,RED]
    gi_f = x_tbd @ np.asarray(inputs["gru_wi_f"], np.float32).T + np.asarray(
        inputs["gru_bi_f"], np.float32
    )
    gi_b = x_tbd @ np.asarray(inputs["gru_wi_b"], np.float32).T + np.asarray(
        inputs["gru_bi_b"], np.float32
    )
    yf = _gru_dir(gi_f, np.asarray(inputs["gru_wh_f"], np.float32),
                  np.asarray(inputs["gru_bh_f"], np.float32), reverse=False)
    yb = _gru_dir(gi_b, np.asarray(inputs["gru_wh_b"], np.float32),
                  np.asarray(inputs["gru_bh_b"], np.float32), reverse=True)
    outputs = np.concatenate([yf, yb], -1).transpose(1, 0, 2)  # [B,T,2H]

    q_w = np.asarray(inputs["q_w"], np.float32)
    q_b = np.asarray(inputs["q_b"], np.float32)
    q = (outputs @ q_w.T + q_b)[..., 0]  # [B,T]

    lengths = inputLength.astype(np.int64) - 2 * (TIME_INTERVAL // 2) - 1
    mask = np.arange(T)[None, :] < lengths[:, None]
    qm = np.where(mask, q, 0.0).astype(np.float32)

    total = np.zeros((B,), np.float32)
    for wk in ("w1", "w2", "w3"):
        w = np.asarray(inputs[wk], np.float32)
        logits = np.where(mask, _conv1d_same(qm, w), NEG).astype(np.float32)
        m = logits.max(-1, keepdims=True)
        e = np.exp(logits - m)
        sm = e / e.sum(-1, keepdims=True)
        total = total + (sm * qm).sum(-1)
    return (total / 3.0)[:, None].astype(np.float32)


# revision 32
# speedup vs baseline: 1.5577x; 1.0934x over previous
"""Trainium2 kernel for nn_LJCH1_34548716929306 (ragged_sequence).

Strategy (pure data-parallel over batch, per sharding hint):
  - The dominant cost is the fc0 projection: concat([featContent,
    featDistort, motionFeat]) [16,2048,4864] @ fc0_w.T [4864,128].
    ~637MB fp32 of activations -> memory-regime. Runs on the 8
    NeuronCores, 2 samples per core, as sT = W^T-packed @ xT with
    feature-major (K-major) bf16 layout prepared host-side.
  - Custom Bass/Tile kernel per core (fp16 operands; fp8 E4M3 was tried
    and fails the 2e-2 accuracy gate at 2.3e-2):
      * fc0 weights packed [128(K-part), 38, 128] resident in SBUF
      * x streamed in 8 chunks of [128, 38, 512] fp16 (~5MB), triple
        buffered; one dma_start per chunk, alternating between the two
        HWDGE rings (sync/scalar); per-partition contiguous HBM layout
        (38KB linear blocks)
      * each chunk load split into k-halves issued concurrently on
        both HWDGE rings; output stores routed via the gpsimd SWDGE
        ring so they never stall the load rings
      * 38-step PSUM fp32 accumulation per 512-col chunk
      * result copied PSUM->SBUF (DVE, fp32->fp16) and DMA'd out fp16
      * in the timing loop, chunk 0 is software-pipelined across the
        back edge (persistent tile refreshed at end of body)
  - `reps` builds the same kernel with the whole body inside a
    hardware For_i loop (identical addresses per iteration; weights
    hoisted out). Used by the test harness to measure steady-state
    per-iteration HW time as a slope, cancelling the axon-tunnel RTT.
  - The BiGRU over T=2048 (H=32) and the masked multi-scale softmax
    head are tiny (~0.1% of FLOPs) and sequential; they run in fp32
    numpy on host.
"""

import numpy as np
import ml_dtypes
from concurrent.futures import ThreadPoolExecutor

import concourse.bass as bass
import concourse.bacc as bacc
import concourse.tile as tile
from concourse import mybir
from concourse.bass import ds, ts
from concourse.bass_utils import run_bass_kernel_spmd

B, T = 16, 2048
D_CONTENT, D_DISTORT, D_MOTION = 4096, 512, 256
D = D_CONTENT + D_DISTORT + D_MOTION  # 4864
RED, H = 128, 32
N_CORES = 8
BL = B // N_CORES  # 2 samples per core
NCOL = BL * T  # 4096 columns per core
KT = D // 128  # 38 K-tiles
CH = 512  # super-chunk columns
N_CH = NCOL // CH  # 8
TIME_INTERVAL = 2
NEG = -1e9

USE_FP8 = False  # fp8 E4M3 fails the 2e-2 gate (2.3e-2); fp16 matches bf16
W_SCALE = 32.0  # only used on the fp8 path

_compiled = {}  # reps -> Bacc


def _build_nc(reps=1, dt_in=None, staggered=False, mode="full",
              out_eng="gpsimd", split_load=True, out_f16=True):
    if dt_in is None:
        dt_in = mybir.dt.float8e4 if USE_FP8 else mybir.dt.float16
    nc = bacc.Bacc(
        "TRN2",
        target_bir_lowering=False,
        debug=False,
        enable_asserts=False,
        num_devices=N_CORES,
    )
    # x4[p, j, k, c] = x[k*128+p, j*CH+c]: per-(partition, chunk) data is
    # one contiguous KT*CH block -> max DMA descriptor efficiency.
    x4 = nc.dram_tensor("x4", [128, N_CH, KT, CH], dt_in, kind="ExternalInput")
    w = nc.dram_tensor("w", [128, KT, RED], dt_in, kind="ExternalInput")
    dt_out = mybir.dt.float16 if out_f16 else mybir.dt.float32
    sT = nc.dram_tensor("sT", [RED, NCOL], dt_out, kind="ExternalOutput")

    with tile.TileContext(nc) as tc:
        with tc.tile_pool(name="wp", bufs=1) as wp, \
             tc.tile_pool(name="x0p", bufs=1) as x0p, \
             tc.tile_pool(name="xp", bufs=3) as xp, \
             tc.tile_pool(name="op", bufs=3) as op, \
             tc.tile_pool(name="pp", bufs=4, space="PSUM") as pp:
            w_sb = wp.tile([128, KT, RED], dt_in)
            nc.sync.dma_start(w_sb[:], w.ap())

            def load_engine(j):
                # alternate chunk loads across both HWDGE rings
                return nc.sync if j % 2 == 0 else nc.scalar

            def load_chunk(xt, j):
                if split_load:
                    # halves of the k-range on both rings concurrently
                    nc.sync.dma_start(xt[:, 0 : KT // 2, :],
                                      x4.ap()[:, j, 0 : KT // 2])
                    nc.scalar.dma_start(xt[:, KT // 2 :, :],
                                        x4.ap()[:, j, KT // 2 :])
                else:
                    load_engine(j).dma_start(xt[:], x4.ap()[:, j])

            def store_engine(j):
                if out_eng == "gpsimd":
                    return nc.gpsimd
                if out_eng == "same":
                    return load_engine(j)
                return nc.sync

            def compute(xt, j):
                ps = pp.tile([128, 512], mybir.dt.float32, tag="ps")
                if dt_in == mybir.dt.float8e4:
                    for k2 in range(KT // 2):
                        nc.tensor.matmul(
                            ps[:],
                            w_sb[:, 2 * k2 : 2 * k2 + 2, :],
                            xt[:, 2 * k2 : 2 * k2 + 2, :],
                            start=(k2 == 0),
                            stop=(k2 == KT // 2 - 1),
                            perf_mode=mybir.MatmulPerfMode.DoubleRow,
                        )
                else:
                    for k in range(KT):
                        nc.tensor.matmul(
                            ps[:],
                            w_sb[:, k, :],
                            xt[:, k : k + 1, :],
                            start=(k == 0),
                            stop=(k == KT - 1),
                        )
                ot = op.tile([128, 512], dt_out, tag="o")
                nc.vector.tensor_copy(ot[:], ps[:])
                store_engine(j).dma_start(sT.ap()[:, ds(j * CH, CH)], ot[:])

            def chunk(j):
                xt = xp.tile([128, KT, CH], dt_in, tag="x")
                load_chunk(xt, j)
                compute(xt, j)

            if mode == "mm":
                # diagnostic: matmuls only, from one resident chunk
                xt0 = x0p.tile([128, KT, CH], dt_in)
                nc.sync.dma_start(xt0[:], x4.ap()[:, 0])
                with tc.For_i(0, reps, hint_engines=(mybir.EngineType.PE,)):
                    for j in range(N_CH):
                        compute(xt0, j)
            elif mode == "dma":
                # diagnostic: chunk loads only + one anchor compute
                with tc.For_i(0, reps, hint_engines=(mybir.EngineType.PE,)):
                    last = None
                    for j in range(N_CH):
                        xt = xp.tile([128, KT, CH], dt_in, tag="x")
                        load_chunk(xt, j)
                        last = xt
                    compute(last, 0)
            elif reps == 1:
                for j in range(N_CH):
                    chunk(j)
            else:
                # Software-pipeline chunk 0 across the back edge: its tile is
                # loaded before the loop and refreshed at the END of each
                # iteration (overlapping chunks 1..N-1), so the iteration
                # never stalls on its first load. Same per-iteration HBM
                # traffic (N_CH chunk loads), no start-of-body DMA ramp.
                xt0 = x0p.tile([128, KT, CH], dt_in)
                nc.sync.dma_start(xt0[:], x4.ap()[:, 0])
                with tc.For_i(0, reps, hint_engines=(mybir.EngineType.PE,)):
                    compute(xt0, 0)
                    for j in range(1, N_CH):
                        chunk(j)
                    nc.sync.dma_start(xt0[:], x4.ap()[:, 0])
    nc.compile()
    return nc


def _get_compiled(reps=1, dt_in=None, staggered=False, mode="full", **kw):
    key = (reps, dt_in, staggered, mode, tuple(sorted(kw.items())))
    if key not in _compiled:
        _compiled[key] = _build_nc(reps, dt_in=dt_in, staggered=staggered,
                                   mode=mode, **kw)
    return _compiled[key]


_runners = {}  # reps -> dict(run=..., sharded=..., mesh=..., names...)


def _make_runner(reps=1, dt_in=None, staggered=False, mode="full", **kw):
    """Build the sharded PJRT executable once and reuse it across calls.

    Uses bass2jax fast-dispatch (no effects -> C++ dispatch path) and no
    donation so the same device-resident buffers can be re-executed.
    """
    import jax
    from jax.sharding import Mesh, PartitionSpec
    from jax.experimental.shard_map import shard_map
    from concourse import bass2jax
    from concourse import mybir as _mybir

    nc = _get_compiled(reps, dt_in=dt_in, staggered=staggered, mode=mode, **kw)
    bass2jax.install_neuronx_cc_hook()

    partition_name = nc.partition_id_tensor.name if nc.partition_id_tensor else None
    in_names, out_names, out_avals = [], [], []
    for alloc in nc.m.functions[0].allocations:
        if not isinstance(alloc, _mybir.MemoryLocationSet):
            continue
        name = alloc.memorylocations[0].name
        if alloc.kind == "ExternalInput":
            if name != partition_name:
                in_names.append(name)
        elif alloc.kind == "ExternalOutput":
            out_names.append(name)
            out_avals.append(
                jax.core.ShapedArray(tuple(alloc.tensor_shape), _mybir.dt.np(alloc.dtype))
            )
    all_in_names = list(in_names) + list(out_names)
    if partition_name is not None:
        all_in_names.append(partition_name)

    def _body(*args):
        operands = list(args)
        if partition_name is not None:
            operands.append(bass2jax.partition_id_tensor())
        outs = bass2jax._bass_exec_p.bind(
            *operands,
            out_avals=tuple(out_avals),
            in_names=tuple(all_in_names),
            out_names=tuple(out_names),
            lowering_input_output_aliases=(),
            sim_require_finite=True,
            sim_require_nnan=True,
            nc=nc,
        )
        return tuple(outs)

    devices = jax.devices()[:N_CORES]
    mesh = Mesh(np.asarray(devices), ("core",))
    n_params, n_outs = len(in_names), len(out_avals)
    in_specs = (PartitionSpec("core"),) * (n_params + n_outs)
    out_specs = (PartitionSpec("core"),) * n_outs

    _np_map = {
        mybir.dt.float8e4: ml_dtypes.float8_e4m3,
        mybir.dt.bfloat16: ml_dtypes.bfloat16,
        mybir.dt.float16: np.float16,
        None: ml_dtypes.float8_e4m3 if USE_FP8 else np.float16,
    }
    np_in = _np_map[dt_in]
    in_shapes = {"x4": (N_CORES * 128, N_CH, KT, CH), "w": (N_CORES * 128, KT, RED)}
    avals = [jax.ShapeDtypeStruct(in_shapes[n], np_in) for n in in_names]
    avals += [
        jax.ShapeDtypeStruct((N_CORES * a.shape[0], *a.shape[1:]), a.dtype)
        for a in out_avals
    ]

    def _jit():
        return jax.jit(
            shard_map(_body, mesh=mesh, in_specs=in_specs, out_specs=out_specs,
                      check_rep=False),
            keep_unused=True,
        )

    try:
        sharded = bass2jax.fast_dispatch_compile(
            lambda: _jit().lower(*avals).compile()
        )
    except Exception:
        sharded = _jit()

    def run(in_maps):
        from jax.sharding import NamedSharding
        sh = NamedSharding(mesh, PartitionSpec("core"))
        concat_in = [
            np.concatenate([np.asarray(m[name]) for m in in_maps], axis=0)
            for name in in_names
        ]
        concat_zeros = [
            np.zeros((N_CORES * a.shape[0], *a.shape[1:]), a.dtype) for a in out_avals
        ]
        out_arrs = sharded(*[jax.device_put(a, sh) for a in concat_in + concat_zeros])
        return [
            {
                name: np.asarray(out_arrs[i]).reshape(N_CORES, *out_avals[i].shape)[c]
                for i, name in enumerate(out_names)
            }
            for c in range(N_CORES)
        ]

    return dict(
        run=run, sharded=sharded, mesh=mesh,
        in_names=in_names, out_names=out_names, out_avals=out_avals,
    )


def _get_runner(reps=1, dt_in=None, staggered=False, mode="full", **kw):
    key = (reps, dt_in, staggered, mode, tuple(sorted(kw.items())))
    if key not in _runners:
        _runners[key] = _make_runner(reps, dt_in=dt_in, staggered=staggered,
                                     mode=mode, **kw)
    return _runners[key]


def _run_device(in_maps):
    try:
        return _get_runner(1)["run"](in_maps)
    except Exception:
        return run_bass_kernel_spmd(_get_compiled(1), in_maps, list(range(N_CORES))).results


def _sigmoid(x):
    return 1.0 / (1.0 + np.exp(-x))


def _gru_dir(gi, wh, bh, reverse):
    # gi: [T, B, 3H] precomputed input gates; returns ys [T, B, H]
    Tn, Bn, _ = gi.shape
    whT = wh.T.copy()  # [H, 3H]
    h = np.zeros((Bn, H), np.float32)
    ys = np.empty((Tn, Bn, H), np.float32)
    order = range(Tn - 1, -1, -1) if reverse else range(Tn)
    for t in order:
        g = gi[t]
        gh = h @ whT + bh
        i_r, i_z, i_n = g[:, :H], g[:, H : 2 * H], g[:, 2 * H :]
        h_r, h_z, h_n = gh[:, :H], gh[:, H : 2 * H], gh[:, 2 * H :]
        r = _sigmoid(i_r + h_r)
        z = _sigmoid(i_z + h_z)
        n = np.tanh(i_n + r * h_n)
        h = (1.0 - z) * n + z * h
        ys[t] = h
    return ys


def _conv1d_same(x, w):
    # cross-correlation with zero 'same' padding; x [B,T], w [k]
    k = w.shape[0]
    p = k // 2
    xp = np.pad(x, ((0, 0), (p, p)))
    out = np.zeros_like(x)
    for j in range(k):
        out += w[j] * xp[:, j : j + x.shape[1]]
    return out


def _pack_weights(fc0_w):
    # w_packed[p, k, m] = fc0_w[m, k*128+p] * scale  (lhsT k-tiles)
    np_in = ml_dtypes.float8_e4m3 if USE_FP8 else np.float16
    scale = W_SCALE if USE_FP8 else 1.0
    return np.ascontiguousarray(
        (fc0_w.T * scale).reshape(KT, 128, RED).transpose(1, 0, 2)
    ).astype(np_in)


def _make_in_maps(inputs):
    fC = np.asarray(inputs["featContent"], np.float32)
    fD = np.asarray(inputs["featDistort"], np.float32)
    mF = np.asarray(inputs["motionFeat"], np.float32)
    fc0_w = np.asarray(inputs["fc0_w"], np.float32)
    w_np = _pack_weights(fc0_w)
    np_in = ml_dtypes.float8_e4m3 if USE_FP8 else np.float16

    def build(c):
        sl = slice(c * BL, (c + 1) * BL)
        xT = np.empty((D, NCOL), np_in)
        xT[:D_CONTENT] = fC[sl].reshape(NCOL, D_CONTENT).T
        xT[D_CONTENT : D_CONTENT + D_DISTORT] = fD[sl].reshape(NCOL, D_DISTORT).T
        xT[D_CONTENT + D_DISTORT :] = mF[sl].reshape(NCOL, D_MOTION).T
        # x4[p, j, k, c] = xT[k*128+p, j*CH+c]
        x4 = np.ascontiguousarray(
            xT.reshape(KT, 128, N_CH, CH).transpose(1, 2, 0, 3)
        )
        return {"x4": x4, "w": w_np}

    with ThreadPoolExecutor(N_CORES) as ex:
        return list(ex.map(build, range(N_CORES)))


def kernel(**inputs):
    inputLength = np.asarray(inputs["inputLength"])
    fc0_b = np.asarray(inputs["fc0_b"], np.float32)

    in_maps = _make_in_maps(inputs)
    results = _run_device(in_maps)

    unscale = np.float32(1.0 / W_SCALE) if USE_FP8 else np.float32(1.0)
    scores = np.empty((B, T, RED), np.float32)
    for c in range(N_CORES):
        sT = results[c]["sT"]  # [RED, BL*T]
        scores[c * BL : (c + 1) * BL] = (
            sT.T.reshape(BL, T, RED).astype(np.float32) * unscale
        )
    scores += fc0_b

    # BiGRU (fp32 host)
    x_tbd = scores.transpose(1, 0, 2)  # [T,B,RED]
    gi_f = x_tbd @ np.asarray(inputs["gru_wi_f"], np.float32).T + np.asarray(
        inputs["gru_bi_f"], np.float32
    )
    gi_b = x_tbd @ np.asarray(inputs["gru_wi_b"], np.float32).T + np.asarray(
        inputs["gru_bi_b"], np.float32
    )
    yf = _gru_dir(gi_f, np.asarray(inputs["gru_wh_f"], np.float32),
                  np.asarray(inputs["gru_bh_f"], np.float32), reverse=False)
    yb = _gru_dir(gi_b, np.asarray(inputs["gru_wh_b"], np.float32),
                  np.asarray(inputs["gru_bh_b"], np.float32), reverse=True)
    outputs = np.concatenate([yf, yb], -1).transpose(1, 0, 2)  # [B,T,2H]

    q_w = np.asarray(inputs["q_w"], np.float32)
    q_b = np.asarray(inputs["q_b"], np.float32)
    q = (outputs @ q_w.T + q_b)[..., 0]  # [B,T]

    lengths = inputLength.astype(np.int64) - 2 * (TIME_INTERVAL // 2) - 1
    mask = np.arange(T)[None, :] < lengths[:, None]
    qm = np.where(mask, q, 0.0).astype(np.float32)

    total = np.zeros((B,), np.float32)
    for wk in ("w1", "w2", "w3"):
        w = np.asarray(inputs[wk], np.float32)
        logits = np.where(mask, _conv1d_same(qm, w), NEG).astype(np.float32)
        m = logits.max(-1, keepdims=True)
        e = np.exp(logits - m)
        sm = e / e.sum(-1, keepdims=True)
        total = total + (sm * qm).sum(-1)
    return (total / 3.0)[:, None].astype(np.float32)
